# revision 12
# baseline (speedup 1.0000x reference)
"""Multi-head attention Trainium2 kernel (8 NeuronCores, SPMD), v3.

Problem: B=4, T=2048, n_feat=512, H=8 heads, d_k=64.
Sharding: core c -> batch b = c//2, head-half hh = c%2 (4 heads = 256 attn dims).

Per-core dataflow (v3 = v2 + fp8 DoubleRow matmuls):
  - Q/K path entirely in fp8e4m3: host ships x_q^T/x_k^T and Wq/Wk in fp8;
    the Q/K projections run as DoubleRow matmuls (2 fo-subtiles per pass,
    0.5 cycles/row) and Q^T/K^T are STORED fp8 as [P, 2, T] tiles holding
    2 heads each (partition 64h + d, sub-1 zero padding) so the scores
    S^T = K^T.T @ Q^T are [64, 2, *] DoubleRow matmuls too -- PE busy drops
    116.6us -> 78.4us vs the all-bf16/f32r v2.  V path stays bf16 (error
    budget: quantization noise enters only through softmax weights, damped
    by the 1/sqrt(dk) scale; measured 1.4e-2 absmax vs 2e-2 gate).
  - Attention j-loop unchanged from v2 (software-pipelined, PV deferred one
    iteration, ACT exp + DVE bf16-Schraudolph FE tiles on the pp slots).
  - xnorm/xT2/Wo/identity in bf16: cheaper PE transposes (1.0 cyc/row) and
    2x-mode DVE copies for the transpose staging.
  - GPSIMD/Pool cannot access PSUM on TRN2 and walrus rejects
    TensorScalarPtr on Pool, so Pool only does SBUF memsets (the fp8
    zero-padding subtiles); every PSUM egress (exp, copies) is ACT/DVE.
  Engines: ACT ~97us busy (pacer), DVE ~88us, PE ~78us; 119.6us total
  (cost model), HW-verified rel err 1.4e-2.
"""
import sys

sys.path.insert(0, "/opt/trn_rl_repo")

import numpy as np
import ml_dtypes

import concourse.bass as bass
import concourse.tile as tile
from concourse import bacc, mybir
from concourse.bass import broadcast_tensor_aps
from concourse.bass_utils import run_bass_kernel_spmd

P = 128
T = 2048
F = 512            # n_feat (projection contraction dim)
OB = 256           # per-core attention dims (4 heads x 64)
NH = 4             # local heads
DK = 64
NT = T // P        # 16 row tiles
FO = F // P        # 4 feature tiles
NSUP = 2           # i-supers
ISUP = T // NSUP   # 1024
JT = NT            # 16 j tiles
NEG = -1.0e30
EPS = 1e-8
DW = DK + 1        # PV output width per i-tile (x + Z)
# bf16-Schraudolph fast-exp constants (DVE offload of some exp tiles):
# int16 bits = round(x * 128/ln2 + (16256 - C)); C calibrated, ~2% rms.
FE_C1 = 128.0 / np.log(2.0) / 8.0      # folds the 1/sqrt(d_k)=1/8 scale
FE_MASKMUL = 128.0 / np.log(2.0)
FE_MAGIC = 16256.0 - 5.25
FE_JTS_D = (3, 7, 11, 14)          # j-tiles computed on DVE instead of ACT

f32 = mybir.dt.float32
f32r = mybir.dt.float32r
bf16 = mybir.dt.bfloat16
fp8 = mybir.dt.float8e4

_CACHE = {}


def _build():
    nc = bacc.Bacc("TRN2", target_bir_lowering=False, debug=False, num_devices=8)

    xqT = nc.dram_tensor("xqT", (F, T), fp8, kind="ExternalInput").ap()
    xkT = nc.dram_tensor("xkT", (F, T), fp8, kind="ExternalInput").ap()
    xvT = nc.dram_tensor("xvT", (F, T), bf16, kind="ExternalInput").ap()
    # wqk: [WqT | WkT] merged; cst: [bqr(2) | bkr(2) | mb(16) | mbc(16) | bvb(256)]
    wqk = nc.dram_tensor("wqk", (F, 2 * OB), fp8, kind="ExternalInput").ap()
    wvT = nc.dram_tensor("wvT", (F, OB), bf16, kind="ExternalInput").ap()
    woT = nc.dram_tensor("woT", (OB, F), bf16, kind="ExternalInput").ap()
    cst = nc.dram_tensor("cst", (P, 292), f32, kind="ExternalInput").ap()
    ident = nc.dram_tensor("ident", (P, P), bf16, kind="ExternalInput").ap()
    outD = nc.dram_tensor("out", (T, F), bf16, kind="ExternalOutput").ap()

    with tile.TileContext(nc) as tc:
        with tc.tile_pool(name="const", bufs=1) as cpool, \
             tc.tile_pool(name="xin", bufs=1) as xpool, \
             tc.tile_pool(name="persist", bufs=1) as ppool, \
             tc.tile_pool(name="et", bufs=8) as epool, \
             tc.tile_pool(name="norm", bufs=2) as npool, \
             tc.tile_pool(name="ost", bufs=8) as opool, \
             tc.tile_pool(name="ps", bufs=2, space="PSUM") as ps, \
             tc.tile_pool(name="psacc", bufs=1, space="PSUM") as psa:

            # ---- warm-up: hoist the activation-table load off the fill path
            warm = cpool.tile([1, 2], f32, tag="warm")
            nc.vector.memset(warm[0:1, 0:1], 0.0)
            nc.scalar.activation(
                warm[0:1, 1:2], warm[0:1, 0:1], mybir.ActivationFunctionType.Exp
            )
            # PE p-state warm-up: a continuous stream of tiny matmuls so the
            # fill-phase projections run at full clock (ramp needs ~3us).
            wb = cpool.tile([1, P], bf16, tag="warmb")
            nc.gpsimd.memset(wb[:], 0.0)
            wup = ps.tile([P, ISUP], f32, tag="big")
            for _ in range(48):
                nc.tensor.matmul(
                    wup[0:1, 0:P], wb[0:1, 0:1], wb[0:1, :], start=True, stop=True
                )

            # ---- constant / weight loads (SP queue), fill-critical first.
            # The sim serializes DMA transfers, so the prefix ships only the
            # bytes the first scores need: biases+mask (36 cols), wq_o0,
            # xq, wk_o0, xk(0:256); bvb and the rest follow.
            cst_sb = cpool.tile([P, 292], f32, tag="cst")
            nc.sync.dma_start(out=cst_sb[:], in_=cst[:])
            bq_sb = cst_sb[:, 0:2]
            bk_sb = cst_sb[:, 2:4]
            mb_sb = cst_sb[:, 4:20]
            bv_sb = cst_sb[:, 36:292]

            # wqk layout: [wq_o0 | wk_o0 | wq_o1 | wk_o1], 128 cols each --
            # the o-tile-0 half loads first (fill-critical), o-tile 1 later.
            wqk_sb = cpool.tile([P, FO, 2 * OB], fp8, tag="wqk")
            wqkr = wqk.rearrange("(fo p) o -> p fo o", p=P)
            nc.sync.dma_start(out=wqk_sb[:, :, 0:OB], in_=wqkr[:, :, 0:OB])

            xq_sb = xpool.tile([P, FO, T], fp8, tag="xq")
            xk_sb = xpool.tile([P, FO, T], fp8, tag="xk")
            xv_sb = xpool.tile([P, FO, T], bf16, tag="xv")
            xkr = xkT.rearrange("(fo p) t -> p fo t", p=P)
            xqr = xqT.rearrange("(fo p) t -> p fo t", p=P)
            xvr = xvT.rearrange("(fo p) t -> p fo t", p=P)

            def xdma(sb, dr, lo, hi):
                nc.sync.dma_start(out=sb[:, :, lo:hi], in_=dr[:, :, lo:hi])

            xdma(xq_sb, xqr, 0, 1024)
            xdma(xk_sb, xkr, 0, 256)
            wv_sb = cpool.tile([P, FO, OB], bf16, tag="wv")
            nc.sync.dma_start(out=wv_sb[:], in_=wvT.rearrange("(fo p) o -> p fo o", p=P))
            xdma(xk_sb, xkr, 256, 512)
            xdma(xv_sb, xvr, 0, 512)
            xdma(xk_sb, xkr, 512, 768)
            xdma(xv_sb, xvr, 512, 768)
            xdma(xk_sb, xkr, 768, 1024)
            xdma(xv_sb, xvr, 768, 1024)
            xdma(xk_sb, xkr, 1024, 1280)
            xdma(xv_sb, xvr, 1024, 1280)
            xdma(xk_sb, xkr, 1280, 1536)
            xdma(xv_sb, xvr, 1280, 1536)
            xdma(xk_sb, xkr, 1536, 1792)
            xdma(xv_sb, xvr, 1536, 1792)
            xdma(xk_sb, xkr, 1792, 2048)
            xdma(xv_sb, xvr, 1792, 2048)
            xdma(xq_sb, xqr, 1024, 1536)
            xdma(xq_sb, xqr, 1536, 2048)
            nc.sync.dma_start(out=wqk_sb[:, :, OB:2 * OB], in_=wqkr[:, :, OB:2 * OB])

            id_sb = cpool.tile([P, P], bf16, tag="ident")
            nc.sync.dma_start(out=id_sb[:], in_=ident[:])
            wo_sb = cpool.tile([P, 2, F], bf16, tag="wo")
            nc.sync.dma_start(out=wo_sb[:], in_=woT.rearrange("(pc p) f -> p pc f", p=P))

            # ---- persistent activations ----
            # fp8 Q^T/K^T for DoubleRow scores: per otile a [P, 2, T] tile
            # holding 2 heads (partition 64h + d); sub 1 is ZERO padding so
            # the [64, 2, *] DoubleRow contraction (=128) sums only dk=64.
            QT0 = ppool.tile([P, 2, T], fp8, tag="QT0")
            QT1 = ppool.tile([P, 2, T], fp8, tag="QT1")
            KT0 = ppool.tile([P, 2, T], fp8, tag="KT0")
            KT1 = ppool.tile([P, 2, T], fp8, tag="KT1")
            QT = [QT0, QT1]
            KT = [KT0, KT1]
            V2 = ppool.tile([P, NT, NH, DW], bf16, tag="V2")
            xnorm = ppool.tile([P, NT, OB], bf16, tag="xnorm")
            xT2 = ppool.tile([P, 2, T], bf16, tag="xT2")
            nc.vector.memset(V2[:, :, :, DK:DW], 1.0)
            # zero the DoubleRow padding subtiles on the idle Pool engine
            # (halves: the first score tiles only gate on the early columns)
            nc.gpsimd.memset(KT[0][:, 1, 0:ISUP], 0.0)
            nc.gpsimd.memset(QT[0][:, 1, 0:ISUP], 0.0)
            nc.gpsimd.memset(KT[0][:, 1, ISUP:T], 0.0)
            nc.gpsimd.memset(QT[0][:, 1, ISUP:T], 0.0)
            nc.gpsimd.memset(KT[1][:, 1, :], 0.0)
            nc.gpsimd.memset(QT[1][:, 1, :], 0.0)

            # ---- projection pieces (interleaved into attention via cbs) ----
            # wqk col base: wq(otile) at otile*256, wk(otile) at otile*256+128
            def qk_proj(wbase, x_sb, bias_sb, dst, otile, lo, hi, eng="dve"):
                pp = ps.tile([P, F], f32, tag="pp")
                for fp in range(FO // 2):
                    # fp8 DoubleRow over fo-pairs: contraction 2x128/pass.
                    nc.tensor.matmul(
                        pp[:, 0:hi - lo],
                        wqk_sb[:, 2 * fp:2 * fp + 2,
                               otile * 2 * P + wbase:otile * 2 * P + wbase + P],
                        x_sb[:, 2 * fp:2 * fp + 2, lo:hi],
                        start=(fp == 0),
                        stop=(fp == FO // 2 - 1),
                        perf_mode=mybir.MatmulPerfMode.DoubleRow,
                    )
                v = nc.vector if eng == "dve" else nc.gpsimd
                v.tensor_scalar_add(
                    dst[otile][:, 0, lo:hi],
                    pp[:, 0:hi - lo],
                    bias_sb[:, otile:otile + 1],
                )

            def v_proj(t):
                pp = ps.tile([P, F], f32, tag="pp")
                for fo in range(FO):
                    nc.tensor.matmul(
                        pp[:, :OB],
                        xv_sb[:, fo, t * P:(t + 1) * P],
                        wv_sb[:, fo, :],
                        start=(fo == 0),
                        stop=(fo == FO - 1),
                    )
                nc.vector.tensor_add(
                    V2[:, t, :, 0:DK],
                    pp[:, :OB].rearrange("p (h d) -> p h d", h=NH),
                    bv_sb[:].rearrange("p (h d) -> p h d", h=NH),
                )

            # ---- normalization (DVE only; one PSUM read decouples WAR) ----
            def emit_norm(su, h, acc):
                nsb = npool.tile([P, 2, 4, DW], f32, tag="nsb")
                nc.vector.tensor_copy(
                    nsb[:],
                    acc[:, :, 0:4 * DW].rearrange("p g (r c) -> p g r c", c=DW),
                )
                # Z >= ~13 always (sum of 2048 exps), so the reference's +eps
                # is a <1e-9 relative correction: skip it.
                rr = npool.tile([P, 8], f32, tag="rr")
                nc.vector.reciprocal(
                    rr[:], nsb[:, :, :, DK].rearrange("p g r -> p (g r)")
                )
                for g in (0, 1):
                    # mul reads only SBUF (nsb/rr) -> legal on the idle Pool
                    dst = xnorm[:, su * 8 + g * 4:su * 8 + (g + 1) * 4,
                                h * DK:(h + 1) * DK]
                    r_ap = rr[:, g * 4:(g + 1) * 4].rearrange("p a -> p a ()")
                    r_b, a_b = broadcast_tensor_aps(r_ap, nsb[:, g, :, 0:DK])
                    nc.gpsimd.tensor_mul(dst, a_b, r_b)

            # last-loop variant: no next loop to protect -> read PSUM directly,
            # per-bank halves so the g0 tail chain starts right after it3.
            def norm_half_direct(su, h, acc, g):
                accv = acc[:, g, 0:4 * DW].rearrange("p (r c) -> p r c", c=DW)
                rr = npool.tile([P, 4], f32, tag="rr")
                nc.vector.reciprocal(rr[:], accv[:, :, DK])
                dst = xnorm[:, su * 8 + g * 4:su * 8 + (g + 1) * 4,
                            h * DK:(h + 1) * DK]
                r_ap = rr[:].rearrange("p a -> p a ()")
                r_b, a_b = broadcast_tensor_aps(r_ap, accv[:, :, 0:DK])
                nc.vector.tensor_mul(dst, a_b, r_b)

            # ---- attention j-loop ----
            def emit_jloop(su, h, cbs, cbs_pre=None, last=False,
                           fe_jts=None, fe_big=()):
                qto = QT[h // 2]
                kto = KT[h // 2]
                hp = 64 * (h % 2)
                isl = su * ISUP
                acc = psa.tile([P, 2, F], f32, tag="acc")

                def scores_mm(dst_ap, jt, cc):
                    # fp8 DoubleRow: [64, 2, *] APs; sub1 of K is zero so
                    # the 128-wide contraction sums only dk=64; 0.5 cyc/row.
                    nc.tensor.matmul(
                        dst_ap,
                        kto[hp:hp + 64, :, jt * P:(jt + 1) * P],
                        qto[hp:hp + 64, :, isl + cc * F:isl + (cc + 1) * F],
                        start=True,
                        stop=True,
                        perf_mode=mybir.MatmulPerfMode.DoubleRow,
                    )

                def scores(jt):
                    if jt in fe_jts:
                        # FE tiles take the two "pp" slots: keeps the "big"
                        # double-buffer free so ACT never waits on the
                        # et->PV->scores chain around a DVE-exp tile.
                        a = ps.tile([P, F], f32, tag="pp")
                        b = ps.tile([P, F], f32, tag="pp")
                        scores_mm(a[:], jt, 0)
                        scores_mm(b[:], jt, 1)
                        return (a, b)
                    st = ps.tile([P, ISUP], f32, tag="big")
                    for cc in range(ISUP // F):
                        scores_mm(st[:, cc * F:(cc + 1) * F], jt, cc)
                    return st

                def emit_pv(et, jt):
                    for it in range(8):
                        # PSUM start zeroes the whole 2KB bank: only the first
                        # packed region per bank starts, only the last stops.
                        nc.tensor.matmul(
                            acc[:, it // 4, (it % 4) * DW:(it % 4) * DW + DW],
                            et[:, it * P:(it + 1) * P],
                            V2[:, jt, h, :],
                            start=(jt == 0 and it % 4 == 0),
                            stop=(jt == JT - 1 and it % 4 == 3),
                        )
                        if last and jt == JT - 1 and it == 3:
                            norm_half_direct(su, h, acc, 0)

                st_prev = scores(0)
                pv_prev = None  # PV deferred one iteration: slack for et/V2
                for jt in range(JT):
                    et = epool.tile([P, ISUP], bf16, tag="et")
                    if jt in fe_jts:
                        # bf16-Schraudolph exp on DVE (ACT is the bottleneck):
                        # int16 bits = (st * C1) + mbc, saturating convert.
                        mcol = cst_sb[:, 20 + jt:21 + jt]
                        for cc, stp in enumerate(st_prev):
                            i1, _ = broadcast_tensor_aps(mcol, stp[:])
                            nc.vector.scalar_tensor_tensor(
                                et[:, cc * F:(cc + 1) * F].bitcast(mybir.dt.int16),
                                stp[:],
                                FE_C1,
                                i1,
                                mybir.AluOpType.mult,
                                mybir.AluOpType.add,
                            )
                    elif jt in fe_big:
                        # Schraudolph on DVE straight from the big slot in ONE
                        # instruction -- no pp slots, so no piece conflicts.
                        mcol = cst_sb[:, 20 + jt:21 + jt]
                        i1, _ = broadcast_tensor_aps(mcol, st_prev[:])
                        nc.vector.scalar_tensor_tensor(
                            et[:].bitcast(mybir.dt.int16),
                            st_prev[:],
                            FE_C1,
                            i1,
                            mybir.AluOpType.mult,
                            mybir.AluOpType.add,
                        )
                    else:
                        nc.scalar.activation(
                            et[:],
                            st_prev[:],
                            mybir.ActivationFunctionType.Exp,
                            bias=mb_sb[:, jt:jt + 1],
                            scale=0.125,
                        )
                    if jt + 1 < JT:
                        st_prev = scores(jt + 1)
                    if cbs_pre:
                        for cb in cbs_pre.get(jt, ()):
                            cb()
                    if pv_prev is not None:
                        emit_pv(*pv_prev)
                    pv_prev = (et, jt)
                    for cb in cbs.get(jt, ()):
                        cb()
                emit_pv(*pv_prev)
                if last:
                    norm_half_direct(su, h, acc, 1)
                else:
                    emit_norm(su, h, acc)

            # ---- x transpose + output projection pieces ----
            def transp(su, pc, half, eng="dve"):
                tp = ps.tile([P, F], bf16, tag="pp")
                for k in range(4):
                    t = su * 8 + half * 4 + k
                    nc.tensor.transpose(
                        tp[:, k * P:(k + 1) * P],
                        xnorm[:, t, pc * P:(pc + 1) * P],
                        id_sb[:],
                    )
                dst = xT2[:, pc, (su * 8 + half * 4) * P:(su * 8 + half * 4) * P + F]
                if eng == "act":
                    nc.scalar.copy(dst, tp[:])
                else:
                    nc.vector.tensor_copy(dst, tp[:])

            def outproj(t, eng="dve", psl=None, osb=None, oi=None):
                pp = psl if psl is not None else ps.tile([P, F], f32, tag="pp")
                for pc in range(2):
                    nc.tensor.matmul(
                        pp[:],
                        xT2[:, pc, t * P:(t + 1) * P],
                        wo_sb[:, pc, :],
                        start=(pc == 0),
                        stop=(pc == 1),
                    )
                if osb is not None:
                    # tail batching: stage into a shared tile; caller DMAs
                    # 4 tiles at once (one HWDGE issue instead of four).
                    dst = osb[:, oi, :]
                else:
                    os = opool.tile([P, F], bf16, tag="os")
                    dst = os[:]
                if eng == "act":
                    nc.scalar.copy(dst, pp[:])
                else:
                    nc.vector.tensor_copy(dst, pp[:])
                if osb is None:
                    nc.sync.dma_start(out=outD[t * P:(t + 1) * P, :], in_=dst)

            # ---- pipeline: prefix proj, then su-major attention with cbs ----
            def K_(ot, lo, hi, eng="dve"):
                return lambda: qk_proj(P, xk_sb, bk_sb, KT, ot, lo, hi, eng)

            def Q_(ot, lo, hi, eng="dve"):
                return lambda: qk_proj(0, xq_sb, bq_sb, QT, ot, lo, hi, eng)

            qk_proj(0, xq_sb, bq_sb, QT, 0, 0, 512)
            qk_proj(P, xk_sb, bk_sb, KT, 0, 0, 256)
            qk_proj(0, xq_sb, bq_sb, QT, 0, 512, 1024)

            # piece slots avoid jt in {2,3,6,7,10,11,13,14}: FE scores
            # (jt in 3,7,11,14) occupy the "pp" slots during jt-1 and jt.
            cbs_pre_list = {
                (0, 0): {0: [K_(0, 256, 512)]},
            }
            cbs_list = {
                (0, 0): {
                    0: [lambda: v_proj(0), lambda: v_proj(1)],
                    1: [lambda: v_proj(2), lambda: v_proj(3), K_(0, 512, 768)],
                    4: [lambda: v_proj(4), lambda: v_proj(5), K_(0, 768, 1024)],
                    5: [lambda: v_proj(6), lambda: v_proj(7), K_(0, 1024, 1280)],
                    8: [lambda: v_proj(8), lambda: v_proj(9), K_(0, 1280, 1536)],
                    9: [lambda: v_proj(10), lambda: v_proj(11),
                        K_(0, 1536, 1792)],
                    12: [lambda: v_proj(12), lambda: v_proj(13),
                         lambda: v_proj(14), K_(0, 1792, 2048)],
                    15: [lambda: v_proj(15)],
                },
                (0, 1): {
                    0: [K_(1, 0, 512)],
                    1: [K_(1, 512, 1024)],
                    4: [K_(1, 1024, 1536)],
                    5: [Q_(1, 0, 512)],
                    8: [Q_(1, 512, 1024)],
                    9: [Q_(0, 1024, 1536)],
                },
                (0, 2): {
                    0: [K_(1, 1536, 2048)],
                    1: [Q_(0, 1536, 2048)],
                    12: [Q_(1, 1024, 1536)],
                    15: [Q_(1, 1536, 2048)],
                },
                (1, 0): {
                    0: [lambda: transp(0, 0, 0)],
                    1: [lambda: transp(0, 0, 1)],
                    4: [lambda: transp(0, 1, 0)],
                    5: [lambda: transp(0, 1, 1)],
                    8: [lambda: outproj(0)],
                    9: [lambda: outproj(1)],
                    12: [lambda: outproj(2)],
                    15: [lambda: outproj(3)],
                },
                (1, 1): {
                    0: [lambda: outproj(4)],
                    1: [lambda: outproj(5)],
                    12: [lambda: outproj(6)],
                    15: [lambda: outproj(7)],
                },
                (1, 3): {
                    0: [lambda: transp(1, 0, 0)],
                    1: [lambda: transp(1, 0, 1)],
                },
            }

            # piece-light loops run more DVE-exp tiles (ACT would otherwise
            # pace them above the PE floor); piece slots stay in {0,1,12,15}.
            fe_map = {
                (0, 0): (),
                (0, 1): (3, 7, 11, 14),
                (0, 2): (3, 5, 7, 9, 11, 14),
                (0, 3): (3, 5, 7, 9, 11, 14),
                (1, 1): (3, 5, 7, 9, 11, 14),
                (1, 2): (3, 5, 7, 9, 11, 13, 14),
                (1, 3): (3, 5, 7, 9, 11, 14),
            }
            fe_big_map = {
                (0, 2): (13,),
                (0, 3): (13,),
                (1, 3): (13,),
            }
            for su in range(NSUP):
                for h in range(NH):
                    emit_jloop(su, h, cbs_list.get((su, h), {}),
                               cbs_pre_list.get((su, h)),
                               last=(su == NSUP - 1 and h == NH - 1),
                               fe_jts=fe_map.get((su, h), FE_JTS_D),
                               fe_big=fe_big_map.get((su, h), ()))

            # tail: pc1/su1 transposes + su1 outproj, copies alternate ACT/DVE;
            # outproj psum via free "big" halves (4 in flight, no slot waits).
            transp(1, 1, 0, "act")
            bslots = [ps.tile([P, ISUP], f32, tag="big", name=f"obig{i}")
                      for i in range(2)]
            outr = outD.rearrange("(t p) f -> p t f", p=P)
            osb8 = opool.tile([P, 4, F], bf16, tag="osb8")
            for i, t in enumerate(range(8, 12)):
                if t == 10:
                    transp(1, 1, 1, "act")
                outproj(t, "act" if i % 2 == 1 else "dve",
                        psl=bslots[i // 2][:, (i % 2) * F:(i % 2 + 1) * F],
                        osb=osb8, oi=i)
            nc.sync.dma_start(out=outr[:, 8:12, :], in_=osb8[:])
            osbC = opool.tile([P, 4, F], bf16, tag="osbC")
            for i, t in enumerate(range(12, 15)):
                outproj(t, "act" if i % 2 == 1 else "dve",
                        psl=bslots[i // 2][:, (i % 2) * F:(i % 2 + 1) * F],
                        osb=osbC, oi=i)
            nc.sync.dma_start(out=outr[:, 12:15, :], in_=osbC[:, 0:3, :])
            # the very last tile ships alone: shortest possible end chain
            outproj(15, "act", psl=bslots[1][:, F:2 * F])

    nc.compile()
    return nc


def _prep_in_maps(query, key, value, mask, Wq, bq, Wk, bk, Wv, bv, Wo):
    bfl = ml_dtypes.bfloat16
    f8l = ml_dtypes.float8_e4m3
    ident = np.eye(P, dtype=np.float32).astype(bfl)
    in_maps = []
    xT_cache = {}
    for b in range(4):
        xT_cache[b] = (
            np.ascontiguousarray(query[b].T).astype(f8l),
            np.ascontiguousarray(key[b].T).astype(f8l),
            np.ascontiguousarray(value[b].T).astype(bfl),
        )
    for c in range(8):
        b = c // 2
        hh = c % 2
        ob = slice(hh * OB, (hh + 1) * OB)
        mbias = np.where(mask[b, 0, :] == 0, np.float32(NEG), np.float32(0.0))
        mbias = np.ascontiguousarray(mbias.reshape(JT, P).T)
        qT, kT, vT = xT_cache[b]
        # cst: [bqr(2) | bkr(2) | mb(16) | mbc(16, reserved) | bvb(256)]
        cst = np.zeros((P, 292), np.float32)
        cst[:, 0:2] = bq[ob].reshape(OB // P, P).T
        cst[:, 2:4] = bk[ob].reshape(OB // P, P).T
        cst[:, 4:20] = mbias
        cst[:, 20:36] = mbias * np.float32(FE_MASKMUL) + np.float32(FE_MAGIC)
        cst[:, 36:292] = bv[ob][None, :]
        in_maps.append({
            "xqT": qT,
            "xkT": kT,
            "xvT": vT,
            # [wq_o0 | wk_o0 | wq_o1 | wk_o1] (128 cols each)
            "wqk": np.ascontiguousarray(np.concatenate([
                Wq[ob, :].T[:, 0:P], Wk[ob, :].T[:, 0:P],
                Wq[ob, :].T[:, P:2 * P], Wk[ob, :].T[:, P:2 * P],
            ], axis=1)).astype(f8l),
            "wvT": np.ascontiguousarray(Wv[ob, :].T).astype(bfl),
            "woT": np.ascontiguousarray(Wo[:, ob].T).astype(bfl),
            "cst": cst,
            "ident": ident,
        })
    return in_maps


def kernel(query, key, value, mask, Wq, bq, Wk, bk, Wv, bv, Wo, bo):
    query = np.asarray(query, dtype=np.float32)
    key = np.asarray(key, dtype=np.float32)
    value = np.asarray(value, dtype=np.float32)
    mask = np.asarray(mask)
    Wq = np.asarray(Wq, dtype=np.float32)
    bq = np.asarray(bq, dtype=np.float32)
    Wk = np.asarray(Wk, dtype=np.float32)
    bk = np.asarray(bk, dtype=np.float32)
    Wv = np.asarray(Wv, dtype=np.float32)
    bv = np.asarray(bv, dtype=np.float32)
    Wo = np.asarray(Wo, dtype=np.float32)
    bo = np.asarray(bo, dtype=np.float32)

    if "nc" not in _CACHE:
        _CACHE["nc"] = _build()
    nc = _CACHE["nc"]

    B = query.shape[0]
    in_maps = _prep_in_maps(query, key, value, mask, Wq, bq, Wk, bk, Wv, bv, Wo)
    res = run_bass_kernel_spmd(nc, in_maps, core_ids=list(range(8)))

    out = np.empty((B, T, F), dtype=np.float32)
    for b in range(B):
        out[b] = (
            res.results[2 * b]["out"].astype(np.float32)
            + res.results[2 * b + 1]["out"].astype(np.float32)
            + bo[None, :]
        )
    return out

